# revision 9
# baseline (speedup 1.0000x reference)
"""Bahdanau additive attention on 8 TRN2 NeuronCores, pure data parallel.

Per core (B_loc = 256 batch rows, two 128-row chunks):
  h1 = features @ W1             -- fp16 matmuls, fp32 PSUM accum
  t  = tanh(h1 + hidden@W2 + b1 + b2)  -- h2/biases folded into PSUM via PE
  scores = t @ Wv                -- fused DVE multiply+reduce
  w  = softmax(scores over S)    -- DVE/ACT
  out = sum_s w * features       -- fused DVE multiply-accumulate

features are cast f32->fp16 during the DMA load (SWDGE) and transposed
on-chip for the matmul stationary operand via the 3D-output xbar DMA
transpose (one instruction per 8-s group).
"""

import numpy as np

import concourse.bass as bass
import concourse.bacc as bacc
import concourse.mybir as mybir
import concourse.tile as tile
from concourse.bass_utils import run_bass_kernel_spmd

F16 = mybir.dt.float16
F32 = mybir.dt.float32
AX = mybir.AxisListType
ALU = mybir.AluOpType
ACTF = mybir.ActivationFunctionType

B, S, E, H, U = 2048, 64, 512, 512, 512
N_CORES = 8
BL = B // N_CORES          # 256 rows per core
NCHUNK = BL // 128         # 2 chunks of 128 rows
S_GRP = 8                  # s rows per cast/transpose group
N_GRP = S // S_GRP
EC = E // 128              # 4 contraction chunks
HC = H // 128

_LAST_RESULTS = {}


def build_kernel(reps: int = 1) -> bacc.Bacc:
    import os
    featt_bufs = int(os.environ.get("FEATT_BUFS", "3"))
    t16_bufs = int(os.environ.get("T16_BUFS", "6"))
    nc = bacc.Bacc(target_bir_lowering=False)

    feat_d = nc.declare_dram_parameter("features", [BL, S, E], F32, isOutput=False)
    hid_d = nc.declare_dram_parameter("hidden", [BL, H], F32, isOutput=False)
    w1_d = nc.declare_dram_parameter("W1", [E, U], F32, isOutput=False)
    b1_d = nc.declare_dram_parameter("b1", [U], F32, isOutput=False)
    w2_d = nc.declare_dram_parameter("W2", [H, U], F32, isOutput=False)
    b2_d = nc.declare_dram_parameter("b2", [U], F32, isOutput=False)
    wv_d = nc.declare_dram_parameter("Wv", [U, 1], F32, isOutput=False)
    id_d = nc.declare_dram_parameter("ident", [128, 128], F16, isOutput=False)
    out_d = nc.declare_dram_parameter("out", [BL, E], F32, isOutput=True)

    with tile.TileContext(nc) as tc:
        with (
            tc.tile_pool(name="const", bufs=1) as cpool,
            tc.tile_pool(name="featn", bufs=2) as fpool,
            tc.tile_pool(name="featT", bufs=featt_bufs) as tpool,
            tc.tile_pool(name="work", bufs=2) as wpool,
            tc.tile_pool(name="tanh", bufs=t16_bufs) as hpool,
            tc.tile_pool(name="ph1", bufs=3, space="PSUM") as ph1,
            tc.tile_pool(name="ph2", bufs=1, space="PSUM") as ph2,
        ):
            # ---- constants / weights (fp16 in SBUF) ----
            w1_sb = cpool.tile([128, EC, U], F16)
            nc.gpsimd.dma_start(w1_sb[:], w1_d.rearrange("(c p) u -> p c u", p=128))
            w2_sb = cpool.tile([128, HC, U], F16)
            nc.gpsimd.dma_start(w2_sb[:], w2_d.rearrange("(c p) u -> p c u", p=128))
            ident = cpool.tile([128, 128], F16)
            nc.sync.dma_start(ident[:], id_d[:])
            ones1 = cpool.tile([1, 128], F16)
            nc.vector.memset(ones1[:], 1.0)
            b1row = cpool.tile([1, U], F16)
            nc.gpsimd.dma_start(b1row[:], b1_d.rearrange("(one u) -> one u", one=1))
            b2row = cpool.tile([1, U], F16)
            nc.gpsimd.dma_start(b2row[:], b2_d.rearrange("(one u) -> one u", one=1))
            wv_row = cpool.tile([1, U], F16)
            nc.gpsimd.dma_start(wv_row[:], wv_d.rearrange("u one -> one u"))

            # wv replicated across partitions via K=1 matmul broadcast
            ps_wv = ph2.tile([128, U], F32, tag="ph2")
            nc.tensor.matmul(ps_wv[:], ones1[:], wv_row[:], start=True, stop=True)
            wv_rep = cpool.tile([128, U], F16)
            nc.scalar.activation(wv_rep[:], ps_wv[:], ACTF.Copy)

            for i, c in enumerate([c for _ in range(reps) for c in range(NCHUNK)]):
                b0 = c * 128
                # ---- h2 = hidden @ W2 + b1 + b2 (fp32 psum) ----
                hid16 = wpool.tile([128, H], F16, name=f"hid16_{i}", tag="hid16")
                nc.gpsimd.dma_start(hid16[:], hid_d[b0:b0 + 128, :])
                hidT = wpool.tile([128, HC, 128], F16, name=f"hidT_{i}", tag="hidT")
                nc.sync.dma_start(hidT[:], hid16[:], transpose=True)
                ps_h2 = ph2.tile([128, U], F32, tag="ph2")
                for k in range(HC):
                    nc.tensor.matmul(
                        ps_h2[:], hidT[:, k, :], w2_sb[:, k, :],
                        start=(k == 0), stop=False,
                    )
                nc.tensor.matmul(ps_h2[:], ones1[:], b1row[:], start=False, stop=False)
                nc.tensor.matmul(ps_h2[:], ones1[:], b2row[:], start=False, stop=True)
                h2_16 = wpool.tile([128, U], F16, name=f"h2_16_{i}", tag="h2_16")
                nc.scalar.activation(h2_16[:], ps_h2[:], ACTF.Copy)

                # ---- features: cast load, one tile per 8-s group (exact deps) ----
                feat16 = []
                for g in range(N_GRP):
                    s0 = g * S_GRP
                    fg = fpool.tile([128, S_GRP, E], F16,
                                    name=f"feat16_{i}_{g}", tag="feat16g",
                                    bufs=2 * N_GRP)
                    nc.gpsimd.dma_start(
                        fg[:], feat_d[b0:b0 + 128, s0:s0 + S_GRP, :])
                    feat16.append(fg)

                scores = wpool.tile([128, S], F32, name=f"scores_{i}", tag="scores")

                # ---- per 8-s group: transpose; per s-pair: matmul+tanh+scores ----
                for g in range(N_GRP):
                    s0 = g * S_GRP
                    featT = tpool.tile([128, S_GRP * EC, 128], F16,
                                       name=f"featT_{i}_{g}", tag="featT")
                    nc.sync.dma_start(featT[:], feat16[g][:], transpose=True)
                    for sp in range(S_GRP // 2):
                        ps = ph1.tile([128, 1024], F32, tag="ph1")
                        for half in range(2):
                            ss = sp * 2 + half          # s index within group
                            col = slice(half * 512, half * 512 + 512)
                            for k in range(EC):
                                nc.tensor.matmul(
                                    ps[:, col],
                                    featT[:, ss * EC + k, :],
                                    w1_sb[:, k, :],
                                    start=(k == 0), stop=False,
                                )
                            nc.tensor.matmul(
                                ps[:, col], ident[:], h2_16[:],
                                start=False, stop=True,
                            )
                        t16 = hpool.tile([128, 1024], F16)
                        nc.scalar.activation(t16[:], ps[:], ACTF.Tanh)
                        for half in range(2):
                            s = s0 + sp * 2 + half
                            dump = hpool.tile([128, 512], F16, tag="dump", bufs=2)
                            # scores[:, s] = sum_u t16 * wv  (TensorTensorReduce
                            # hangs on HW; TensorScalarPtr's accum_out works)
                            nc.vector.scalar_tensor_tensor(
                                out=dump[:],
                                in0=t16[:, half * 512: half * 512 + 512],
                                scalar=1.0,
                                in1=wv_rep[:],
                                op0=ALU.mult, op1=ALU.mult,
                                accum_out=scores[:, s:s + 1],
                            )

                # ---- softmax over s ----
                negmax = wpool.tile([128, 1], F32)
                nc.vector.tensor_reduce(
                    out=negmax[:], in_=scores[:], axis=AX.X, op=ALU.max, negate=True,
                )
                probs = wpool.tile([128, S], F32)
                zsum = wpool.tile([128, 1], F32)
                nc.scalar.activation(
                    probs[:], scores[:], ACTF.Exp,
                    bias=negmax[:], scale=1.0, accum_out=zsum[:],
                )
                rz = wpool.tile([128, 1], F32)
                nc.vector.reciprocal(rz[:], zsum[:])
                wsm = wpool.tile([128, S], F32)
                nc.vector.tensor_scalar_mul(wsm[:], probs[:], rz[:])

                # ---- context = sum_s w[:, s] * feat16[:, s, :] ----
                ctx16 = wpool.tile([128, E], F16, name=f"ctx16_{i}", tag="ctx16")
                nc.vector.memset(ctx16[:], 0.0)
                for s in range(S):
                    nc.vector.scalar_tensor_tensor(
                        out=ctx16[:], in0=feat16[s // S_GRP][:, s % S_GRP, :],
                        scalar=wsm[:, s:s + 1],
                        in1=ctx16[:], op0=ALU.mult, op1=ALU.add,
                    )
                nc.gpsimd.dma_start(out_d[b0:b0 + 128, :], ctx16[:])

    nc.compile()
    return nc


def kernel(**inputs) -> np.ndarray:
    features = np.ascontiguousarray(np.asarray(inputs["features"], dtype=np.float32))
    hidden = np.ascontiguousarray(np.asarray(inputs["hidden"], dtype=np.float32))
    W1 = np.ascontiguousarray(np.asarray(inputs["W1"], dtype=np.float32))
    b1 = np.ascontiguousarray(np.asarray(inputs["b1"], dtype=np.float32))
    W2 = np.ascontiguousarray(np.asarray(inputs["W2"], dtype=np.float32))
    b2 = np.ascontiguousarray(np.asarray(inputs["b2"], dtype=np.float32))
    Wv = np.ascontiguousarray(np.asarray(inputs["Wv"], dtype=np.float32))
    # bv shifts every score equally; softmax is invariant to it.

    nc = build_kernel()
    ident = np.eye(128, dtype=np.float16)
    in_maps = []
    for i in range(N_CORES):
        in_maps.append({
            "features": features[i * BL:(i + 1) * BL],
            "hidden": hidden[i * BL:(i + 1) * BL],
            "W1": W1, "b1": b1, "W2": W2, "b2": b2, "Wv": Wv,
            "ident": ident,
        })
    res = run_bass_kernel_spmd(nc, in_maps, core_ids=list(range(N_CORES)))
    _LAST_RESULTS["res"] = res
    if res.exec_time_ns is not None:
        print(f"HW exec time: {res.exec_time_ns} ns")
    out = np.concatenate([res.results[i]["out"] for i in range(N_CORES)], axis=0)
    return out.astype(np.float32)
